# revision 3
# baseline (speedup 1.0000x reference)
"""TRN2 Bass kernel for GNN message passing (nn_MessagePassing):

    out = segment_sum(x[src] * edge_weight, dst, num_segments=N)

x: [50000, 64] f32, edge_weight: [1250000] f32, edge_index: [2, 1250000] i64.

Distribution strategy (8 NeuronCores, SPMD):
  - Destination nodes are sharded across the 8 cores (core k owns output rows
    [k*6250, (k+1)*6250)), so no all-reduce is needed: each core computes a
    disjoint output slice and the host concatenates them.
  - x is replicated (gathered from HBM on every core).

Per-core device pipeline:
  - Host buckets edges by (core, 64-node dst window, src half) and pads each
    bucket to a multiple of 128 edges ("chunks").
  - dma_gather bulk-gathers the source rows for up to 8 chunks (1024 edges)
    per instruction (int16 indices force splitting x into two <32768-row
    tables).
  - The edge weight is applied with one broadcasted vector multiply per group
    of windows; a one-hot scatter matrix S (S[e, j] = (dst_local[e] == j)) is
    built with an iota==dst compare.
  - out_window += S_chunk^T @ msg_chunk accumulates in PSUM on the tensor
    engine; finished windows are copied to SBUF and DMA'd to the output.
"""

import sys

if "/opt/trn_rl_repo" not in sys.path:
    sys.path.insert(0, "/opt/trn_rl_repo")

import numpy as np

import concourse.bass as bass
import concourse.bacc as bacc
import concourse.mybir as mybir
import concourse.tile as tile
from concourse.bass_utils import run_bass_kernel_spmd

N_CORES = 8
F = 64
W = 64            # dst-window width (one-hot matmul M dim)
CHUNK = 128       # edges per matmul (K dim)
HALF = 25000      # x table split so gather indices fit int16
GROUP_CHUNKS = 48  # max chunks per window-group (SBUF budget)
BIG_BUFS = 4       # buffering depth for msg/S tiles


def _cdiv(a, b):
    return (a + b - 1) // b


def _host_prep(x, edge_weight, edge_index):
    N = x.shape[0]
    npc = N // N_CORES
    nw = _cdiv(npc, W)
    src = np.asarray(edge_index[0]).astype(np.int64)
    dst = np.asarray(edge_index[1]).astype(np.int64)
    wgt = np.asarray(edge_weight).astype(np.float32)
    E = src.shape[0]

    core = dst // npc
    rel = dst - core * npc
    win = rel // W
    dstl = (rel % W).astype(np.float32)
    half = (src >= HALF).astype(np.int64)
    key = (core * nw + win) * 2 + half
    order = np.argsort(key, kind="stable")
    key_s = key[order]
    src_s = src[order]
    dstl_s = dstl[order]
    wgt_s = wgt[order]

    nbuckets = N_CORES * nw * 2
    counts = np.bincount(key_s, minlength=nbuckets).reshape(N_CORES, nw, 2)
    nchunks = _cdiv(counts, CHUNK).max(axis=0)      # [nw, 2], SPMD-uniform
    nchunks[:, 0] = np.maximum(nchunks[:, 0], 1)

    groups = []
    w0, acc = 0, 0
    for w in range(nw):
        t = int(nchunks[w].sum())
        if acc + t > GROUP_CHUNKS and acc > 0:
            groups.append((w0, w))
            w0, acc = w, 0
        acc += t
    groups.append((w0, nw))

    lo_col = np.zeros(nw, np.int64)
    hi_col = np.zeros(nw, np.int64)
    gmeta = []
    c = 0
    for (ws, we) in groups:
        c0 = c
        n_lo = int(nchunks[ws:we, 0].sum())
        n_hi = int(nchunks[ws:we, 1].sum())
        cc = c0
        for w in range(ws, we):
            lo_col[w] = cc
            cc += nchunks[w, 0]
        for w in range(ws, we):
            hi_col[w] = cc
            cc += nchunks[w, 1]
        c = cc
        gmeta.append((c0, n_lo, n_hi, ws, we))
    K_PAD = c

    bstart = np.concatenate([[0], np.cumsum(counts.reshape(-1))])
    slot_base = np.zeros(nbuckets, np.int64)
    for ci in range(N_CORES):
        for w in range(nw):
            slot_base[(ci * nw + w) * 2 + 0] = lo_col[w] * CHUNK
            slot_base[(ci * nw + w) * 2 + 1] = hi_col[w] * CHUNK
    rank = np.arange(E) - bstart[key_s]
    slot = slot_base[key_s] + rank

    x32 = np.ascontiguousarray(np.asarray(x, dtype=np.float32))
    x_lo = x32[:HALF]
    x_hi = np.ascontiguousarray(x32[HALF:])
    iota = np.tile(np.arange(F, dtype=np.float32)[None, :], (128, 1))

    in_maps = []
    for ci in range(N_CORES):
        lo = np.searchsorted(key_s, ci * nw * 2, "left")
        hi_ = np.searchsorted(key_s, (ci + 1) * nw * 2, "left")
        sl = slot[lo:hi_]
        s_src = src_s[lo:hi_]
        idx_stream = np.zeros(K_PAD * CHUNK, np.int32)
        idx_stream[sl] = np.where(s_src >= HALF, s_src - HALF, s_src)
        dstl_stream = np.zeros(K_PAD * CHUNK, np.float32)
        dstl_stream[sl] = dstl_s[lo:hi_]
        wgt_stream = np.zeros(K_PAD * CHUNK, np.float32)
        wgt_stream[sl] = wgt_s[lo:hi_]

        idx16 = np.tile(
            idx_stream.astype(np.int16).reshape(K_PAD * 8, 16).T, (8, 1))
        in_maps.append({
            "x_lo": x_lo, "x_hi": x_hi, "iota": iota,
            "idx16": np.ascontiguousarray(idx16),
            "dstl": dstl_stream.reshape(K_PAD, CHUNK).T.copy(),
            "wgt": wgt_stream.reshape(K_PAD, CHUNK).T.copy(),
        })

    meta = dict(N=N, npc=npc, nw=nw, K_PAD=K_PAD,
                nchunks=nchunks, lo_col=lo_col, hi_col=hi_col, gmeta=gmeta)
    return in_maps, meta


def _build_program(meta, reps=1):
    npc, nw, K_PAD = meta["npc"], meta["nw"], meta["K_PAD"]
    nchunks, lo_col, hi_col = meta["nchunks"], meta["lo_col"], meta["hi_col"]
    gmeta = meta["gmeta"]
    f32, i16 = mybir.dt.float32, mybir.dt.int16

    nc = bacc.Bacc("TRN2", target_bir_lowering=False, debug=False,
                   num_devices=N_CORES, num_swdge_queues=4)
    x_lo_d = nc.dram_tensor("x_lo", [HALF, F], f32, kind="ExternalInput")
    x_hi_d = nc.dram_tensor("x_hi", [meta["N"] - HALF, F], f32,
                            kind="ExternalInput")
    iota_d = nc.dram_tensor("iota", [128, F], f32, kind="ExternalInput")
    idx_d = nc.dram_tensor("idx16", [128, K_PAD * 8], i16,
                           kind="ExternalInput")
    dstl_d = nc.dram_tensor("dstl", [128, K_PAD], f32, kind="ExternalInput")
    wgt_d = nc.dram_tensor("wgt", [128, K_PAD], f32, kind="ExternalInput")
    out_d = nc.dram_tensor("out", [npc, F], f32, kind="ExternalOutput")

    with tile.TileContext(nc) as tc:
        with (
            tc.tile_pool(name="aux", bufs=1) as aux,
            tc.tile_pool(name="big", bufs=BIG_BUFS) as big,
            tc.tile_pool(name="opool", bufs=3) as opool,
            tc.tile_pool(name="psum", bufs=2, space="PSUM") as pp,
        ):
            iota_t = aux.tile([128, F], f32)
            nc.sync.dma_start(out=iota_t[:], in_=iota_d.ap()[:])
            idx_t = aux.tile([128, K_PAD * 8], i16)
            nc.sync.dma_start(out=idx_t[:], in_=idx_d.ap()[:])
            dstl_t = aux.tile([128, K_PAD], f32)
            nc.sync.dma_start(out=dstl_t[:], in_=dstl_d.ap()[:])
            wgt_t = aux.tile([128, K_PAD], f32)
            nc.sync.dma_start(out=wgt_t[:], in_=wgt_d.ap()[:])

            qctr = [0]

            def gather_range(msg_t, table_ap, c0, cstart, nch):
                done = 0
                while done < nch:
                    blk = min(8, nch - done)   # dma_gather limit: 1024 idx
                    cg = cstart + done
                    cl = cg - c0
                    nc.gpsimd.dma_gather(
                        out_ap=msg_t[:, cl * F:(cl + blk) * F].rearrange(
                            "p (c f) -> p c f", f=F),
                        in_ap=table_ap,
                        idxs_ap=idx_t[:, cg * 8:(cg + blk) * 8],
                        num_idxs=blk * CHUNK,
                        num_idxs_reg=blk * CHUNK,
                        elem_size=F,
                        queue_num=qctr[0] % 4,
                    )
                    qctr[0] += 1
                    done += blk

            def body():
                for (c0, n_lo, n_hi, ws, we) in gmeta:
                    kg = n_lo + n_hi
                    msg_t = big.tile([128, kg * F], f32, tag="msg")
                    gather_range(msg_t, x_lo_d.ap()[:], c0, c0, n_lo)
                    if n_hi:
                        gather_range(msg_t, x_hi_d.ap()[:], c0, c0 + n_lo,
                                     n_hi)
                    nc.vector.tensor_tensor(
                        out=msg_t[:], in0=msg_t[:],
                        in1=wgt_t[:, c0:c0 + kg].unsqueeze(2).to_broadcast(
                            [128, kg, F]),
                        op=mybir.AluOpType.mult)
                    S_t = big.tile([128, kg * F], f32, tag="S")
                    nc.vector.tensor_tensor(
                        out=S_t[:],
                        in0=iota_t[:].unsqueeze(1).to_broadcast([128, kg, F]),
                        in1=dstl_t[:, c0:c0 + kg].unsqueeze(2).to_broadcast(
                            [128, kg, F]),
                        op=mybir.AluOpType.is_equal)
                    for w in range(ws, we):
                        cols = (
                            list(range(int(lo_col[w]),
                                       int(lo_col[w] + nchunks[w, 0])))
                            + list(range(int(hi_col[w]),
                                         int(hi_col[w] + nchunks[w, 1]))))
                        ps = pp.tile([W, F], f32, tag="ps")
                        for j, cg in enumerate(cols):
                            cc = cg - c0
                            nc.tensor.matmul(
                                out=ps[:],
                                lhsT=S_t[:, cc * F:(cc + 1) * F],
                                rhs=msg_t[:, cc * F:(cc + 1) * F],
                                start=(j == 0), stop=(j == len(cols) - 1))
                        o_t = opool.tile([W, F], f32, tag="outt")
                        nc.vector.tensor_copy(o_t[:], ps[:])
                        rows = min(W, npc - w * W)
                        nc.sync.dma_start(
                            out=out_d.ap()[w * W:w * W + rows, :],
                            in_=o_t[:rows, :])

            for _ in range(reps):
                body()
    nc.compile()
    return nc


def build_for_inputs(x, edge_weight, edge_index, reps=1):
    """Exposed for test harnesses: returns (nc, in_maps, meta)."""
    in_maps, meta = _host_prep(x, edge_weight, edge_index)
    nc = _build_program(meta, reps=reps)
    return nc, in_maps, meta


def kernel(x, edge_weight, edge_index):
    x = np.asarray(x)
    nc, in_maps, _meta = build_for_inputs(x, edge_weight, edge_index)
    res = run_bass_kernel_spmd(nc, in_maps, core_ids=list(range(N_CORES)))
    out = np.concatenate(
        [res.results[c]["out"] for c in range(N_CORES)], axis=0)
    return out.astype(np.float32)
